# revision 1
# baseline (speedup 1.0000x reference)
"""Trainium2 Bass kernel for nn_Block (dense transformer block with smeared-key
attention and learned cumulative relative positions).

Sharding: tensor-parallel over heads (2 heads/core x 8 cores) for the input
LN + fused projection + attention; AllToAll exchange of z = silu(p) * o / D
(token-resharding); then each core runs the out-projection + final LN for its
256-token slice. Host gathers the 8 slices.

Attention math: scores are built transposed (S^T[j,i] = k~_j . q~_i) so the
probability tiles come out of the QK matmul already in the [j, i] layout the
AV matmul needs (no transposes), the relative-position bias pos_j - pos_i is
added exactly in PSUM via a K=4 rank-2 init matmul (hi/lo split of pos so the
reduced-precision f32r weights carry it exactly), the softmax max-subtraction
uses a per-head Cauchy-Schwarz upper bound c >= max|S| (denominators stay
>= e^-60, no overflow, no cross-tile reduction), row sums come from an M=1
ones matmul, and the 1/D normalization is applied after broadcasting via a
K=1 ones matmul.
"""

import os
import sys
import numpy as np

for _p in ("/opt/trn_rl_repo", "/root/.axon_site/_ro/trn_rl_repo"):
    if os.path.isdir(_p) and _p not in sys.path:
        sys.path.append(_p)

# ---- problem constants (hardcoded per contract) ----
HEADS = 16
D_MODEL = 1024
D_EXP = 2048
D_HEAD = 128
SEQ = 2048
LN_EPS = 1e-5
NC = 8           # cores
HPC = 2          # heads per core
P = 128
NT = SEQ // P    # 16 token tiles
KF = D_MODEL // P  # 8 feature tiles
NCH = 4          # 512-token chunks
IC = 512
TS = SEQ // NC   # 256 tokens per core output slice

_CACHE = {}


def _build_program(use_f32r=True):
    import concourse.bass as bass
    import concourse.mybir as mybir
    import concourse.tile as tile
    from concourse import bacc
    from concourse.bass import _add_dep_helper as add_dep

    f32 = mybir.dt.float32
    fmm = mybir.dt.float32r if use_f32r else mybir.dt.float32
    AF = mybir.ActivationFunctionType
    OP = mybir.AluOpType

    nc = bacc.Bacc("TRN2", target_bir_lowering=False, debug=False,
                   enable_asserts=False, num_devices=NC)

    # ---- DRAM I/O ----
    x_d = nc.dram_tensor("x", [SEQ, D_MODEL], f32, kind="ExternalInput")
    weff_d = nc.dram_tensor("weff", [D_MODEL, 8 * P + 2], fmm, kind="ExternalInput")
    beff_d = nc.dram_tensor("beff", [P, 9], f32, kind="ExternalInput")
    sm_d = nc.dram_tensor("sm", [P, 4], f32, kind="ExternalInput")
    wot_d = nc.dram_tensor("wot", [D_EXP, D_MODEL], fmm, kind="ExternalInput")
    wln_d = nc.dram_tensor("wln", [P, D_MODEL], f32, kind="ExternalInput")
    bln_d = nc.dram_tensor("bln", [P, D_MODEL], f32, kind="ExternalInput")
    mask_d = nc.dram_tensor("masktri", [P, P], f32, kind="ExternalInput")
    ident_d = nc.dram_tensor("ident", [P, P], f32, kind="ExternalInput")
    dsel_d = nc.dram_tensor("dsel", [P, P], fmm, kind="ExternalInput")
    rsel_d = nc.dram_tensor("rsel", [P, P], fmm, kind="ExternalInput")
    cpad_d = nc.dram_tensor("cpad", [P, SEQ], fmm, kind="ExternalInput")
    out_d = nc.dram_tensor("out", [TS, D_MODEL], f32, kind="ExternalOutput")

    C_ROUND = float(3 * (1 << 23))  # fp32 add-magic: rounds to multiples of 2

    with tile.TileContext(nc) as tc:
        with tc.tile_pool(name="const", bufs=1) as const, \
             tc.tile_pool(name="dram", bufs=1, space="DRAM") as dram:

            # ---- small constants ----
            ident = const.tile([P, P], f32, tag="ident", name="ident")
            nc.sync.dma_start(ident[:], ident_d.ap())
            mask = const.tile([P, P], f32, tag="mask", name="mask")
            nc.sync.dma_start(mask[:], mask_d.ap())
            beff = const.tile([P, 9], f32, tag="beff", name="beff")
            nc.sync.dma_start(beff[:], beff_d.ap())
            sm = const.tile([P, 4], f32, tag="sm", name="sm")
            nc.sync.dma_start(sm[:], sm_d.ap())
            dsel = const.tile([P, P], fmm, tag="dsel", name="dsel")
            nc.sync.dma_start(dsel[:], dsel_d.ap())
            rsel = const.tile([P, P], fmm, tag="rsel", name="rsel")
            nc.sync.dma_start(rsel[:], rsel_d.ap())
            epsc = const.tile([P, 1], f32, tag="epsc", name="epsc")
            nc.vector.memset(epsc[:], LN_EPS)

            # ---- DRAM bounce buffers for the per-head AllToAlls ----
            zin = [dram.tile([NC, P, TS], fmm, tag=f"zin{h}", name=f"zin{h}")
                   for h in range(HPC)]
            zout = [dram.tile([NC, P, TS], fmm, tag=f"zout{h}", name=f"zout{h}")
                    for h in range(HPC)]

            # persistent tensors that live from projection through attention
            pers_cm = tc.tile_pool(name="persist", bufs=1)
            persist = pers_cm.__enter__()  # closed at the end (LIFO)
            q_sb = [persist.tile([P, SEQ], fmm, tag=f"q{h}", name=f"q{h}")
                    for h in range(HPC)]
            kt_sb = [persist.tile([P, SEQ], fmm, tag=f"kt{h}", name=f"kt{h}")
                     for h in range(HPC)]
            vT_sb = [persist.tile([P, NT, P], fmm, tag=f"vT{h}", name=f"vT{h}")
                     for h in range(HPC)]
            # p_sb holds silu(p) directly (fused in the projection epilogue)
            p_sb = [persist.tile([P, SEQ], f32, tag=f"p{h}", name=f"p{h}")
                    for h in range(HPC)]
            # pos staging per head: posL rows [hi, lo, 1, 1] (QK-init lhsT),
            # posR rows [1, 1, -hi, -lo] (QK-init rhs)
            # K-padded to 128 rows (rows 4+ zero) so the pos-bias init
            # matmuls are full-array ops (HAM counts array activity)
            posL = [persist.tile([P, SEQ], fmm, tag=f"posL{h}", name=f"posL{h}")
                    for h in range(HPC)]
            posR = [persist.tile([P, SEQ], fmm, tag=f"posR{h}", name=f"posR{h}")
                    for h in range(HPC)]
            cbias = [persist.tile([P, 1], f32, tag=f"cbias{h}", name=f"cbias{h}")
                     for h in range(HPC)]

            # ========== stages A-C: LN, transpose, projection, prep ==========
            with tc.tile_pool(name="weffp", bufs=1) as weffp, \
                 tc.tile_pool(name="stat", bufs=3) as stat, \
                 tc.tile_pool(name="stgB", bufs=1) as stgB, \
                 tc.tile_pool(name="chs", bufs=2) as chs, \
                 tc.tile_pool(name="xcT", bufs=1) as xcTp, \
                 tc.tile_pool(name="psA", bufs=2, space="PSUM") as psA, \
                 tc.tile_pool(name="psY", bufs=1, space="PSUM") as psY, \
                 tc.tile_pool(name="psTP", bufs=4, space="PSUM") as psTP:

                y_sb = stgB.tile([HPC, SEQ], f32, tag="y", name="y")
                bnd = [stgB.tile([P, NCH], f32, tag=f"bnd{h}", name=f"bnd{h}")
                       for h in range(HPC)]

                # ---- stage A: load x, layernorm (streamed, in-place) ----
                # DMA issue order controls queue fair-sharing: first chunk's
                # x tiles, then weff, then the remaining x tiles.
                xp_cm = tc.tile_pool(name="xp", bufs=10)
                xp = xp_cm.__enter__()
                xts = []
                weff = []
                xdmas = []
                for tt in range(NT):
                    xt = xp.tile([P, D_MODEL], f32, tag="x", name=f"x{tt}")
                    xdmas.append(nc.sync.dma_start(
                        xt[:], x_d.ap()[tt * P:(tt + 1) * P, :]))
                    xts.append(xt)
                    if tt == 3:
                        for kf in range(KF):
                            w = weffp.tile([P, 8 * P + 2], fmm,
                                           tag=f"weff{kf}", name=f"weff{kf}")
                            nc.sync.dma_start(
                                w[:], weff_d.ap()[kf * P:(kf + 1) * P, :])
                            weff.append(w)
                for tt in range(NT):
                    xt = xts[tt]
                    bs = stat.tile([P, 12], f32, tag="bs", name="bs")
                    nc.vector.bn_stats(bs[:, 0:6], xt[:, 0:512])
                    nc.vector.bn_stats(bs[:, 6:12], xt[:, 512:1024])
                    mv = stat.tile([P, 2], f32, tag="mv", name="mv")
                    nc.vector.bn_aggr(mv[:], bs[:])
                    rs = stat.tile([P, 1], f32, tag="rs", name="rs")
                    nc.scalar.activation(rs[:], mv[:, 1:2], AF.Sqrt, bias=epsc[:])
                    nc.vector.reciprocal(rs[:], rs[:])
                    # in-place normalize: xt <- (xt - mu) * rstd
                    nc.vector.tensor_scalar(xt[:], xt[:], mv[:, 0:1], rs[:],
                                            OP.subtract, OP.mult)
                nrm = stat.tile([1, 2 * HPC * NCH], f32, tag="nrm",
                                name="nrm", bufs=1)

                # ---- stage B: per-chunk transpose + projection + epilogues ----
                for n in range(NCH):
                    nsl = slice(n * IC, (n + 1) * IC)
                    xcTn = []
                    for kf in range(KF):
                        xT = xcTp.tile([P, IC], fmm, tag=f"xcT{kf}",
                                       name=f"xcT{kf}")
                        for tti in range(4):
                            tt = 4 * n + tti
                            tp = psTP.tile([P, P], f32, tag="tp", name="tp")
                            nc.tensor.transpose(
                                tp[:], xts[tt][:, kf * P:(kf + 1) * P], ident[:])
                            nc.scalar.copy(
                                xT[:, tti * P:(tti + 1) * P], tp[:])
                        xcTn.append(xT)
                    for m in (8, 0, 1, 2, 3, 6, 7, 4, 5):
                        if m < 8:
                            pp = psA.tile([P, IC], f32, tag="pp", name="pp")
                        else:
                            pp = psY.tile([HPC, IC], f32, tag="ypp", name="ypp")
                        for kf in range(KF):
                            if m < 8:
                                lhsT = weff[kf][:, m * P:(m + 1) * P]
                            else:
                                lhsT = weff[kf][:, 8 * P:8 * P + HPC]
                            nc.tensor.matmul(pp[:], lhsT, xcTn[kf][:],
                                             start=(kf == 0), stop=(kf == KF - 1))
                        h = m % 2
                        if m < 2:      # q
                            nc.vector.tensor_scalar_add(q_sb[h][:, nsl], pp[:],
                                                        beff[:, m:m + 1])
                        elif m < 4:    # k: bias, then smear into kt_sb
                            kc = chs.tile([P, IC], f32, tag="kch", name="kch")
                            nc.vector.tensor_scalar_add(kc[:], pp[:],
                                                        beff[:, m:m + 1])
                            ksm = chs.tile([P, IC], f32, tag="ksm", name="ksm", bufs=1)
                            # kt = (1-s)*k ; += s*k shifted right by one
                            nc.vector.tensor_scalar(
                                kt_sb[h][:, nsl], kc[:],
                                sm[:, 2 * h + 1:2 * h + 2], None, OP.mult)
                            nc.vector.tensor_scalar(
                                ksm[:], kc[:], sm[:, 2 * h:2 * h + 1], None,
                                OP.mult)
                            nc.vector.tensor_tensor(
                                kt_sb[h][:, n * IC + 1:(n + 1) * IC],
                                kt_sb[h][:, n * IC + 1:(n + 1) * IC],
                                ksm[:, 0:IC - 1], OP.add)
                            nc.vector.tensor_copy(bnd[h][:, n:n + 1],
                                                  ksm[:, IC - 1:IC])
                            if n > 0:
                                nc.vector.tensor_tensor(
                                    kt_sb[h][:, n * IC:n * IC + 1],
                                    kt_sb[h][:, n * IC:n * IC + 1],
                                    bnd[h][:, n - 1:n], OP.add)
                        elif m < 6:    # v: bias then transpose blocks right away
                            vv = chs.tile([P, IC], f32, tag="vch", name="vch")
                            nc.vector.tensor_scalar_add(vv[:], pp[:],
                                                        beff[:, m:m + 1])
                            for tti in range(4):
                                tp = psTP.tile([P, P], f32, tag="tp", name="tp")
                                nc.tensor.transpose(
                                    tp[:], vv[:, tti * P:(tti + 1) * P], ident[:])
                                nc.scalar.copy(
                                    vT_sb[h][:, 4 * n + tti, :], tp[:])
                        elif m < 8:    # p: fused silu(p + bias)
                            nc.scalar.activation(p_sb[h][:, nsl], pp[:],
                                                 AF.Silu, bias=beff[:, m:m + 1])
                        else:          # y
                            ye = nc.vector.tensor_scalar_add(
                                y_sb[:, nsl], pp[:], beff[0:HPC, 8:9])
                            if n == 1:
                                anchor_mid = ye
                    # per-chunk |q|^2 / |k~|^2 column sums, inline so the
                    # proj->attention boundary has no PE-idle gap
                    for h in range(HPC):
                        for which, src_t in ((0, q_sb[h]), (1, kt_sb[h])):
                            sq2 = chs.tile([P, IC], fmm, tag="sq2", name="sq2",
                                           bufs=1)
                            nc.vector.tensor_tensor(sq2[:], src_t[:, nsl],
                                                    src_t[:, nsl], OP.mult)
                            npp = psY.tile([P, IC], f32, tag="npp", name="npp")
                            nc.tensor.matmul(npp[:], dsel[:], sq2[:],
                                             start=True, stop=True)
                            idx = (h * 2 + which) * NCH + n
                            nc.vector.tensor_reduce(
                                nrm[:, idx:idx + 1], npp[0:1, :],
                                axis=mybir.AxisListType.X, op=OP.max)

                xp_cm.__exit__(None, None, None)
                posw_cm = tc.tile_pool(name="posw", bufs=1)
                posw = posw_cm.__enter__()
                # ---- stage C: c-bound first (it gates the first exp),
                # then the pos staging chain; high_priority interleaves these
                # ops ahead of the tail of the projection work
                with tc.high_priority(offset=150):
                    mx = stat.tile([1, 2 * HPC], f32, tag="mx", name="mx")
                    for h in range(HPC):
                        for which in range(2):
                            base = (h * 2 + which) * NCH
                            nc.vector.tensor_reduce(
                                mx[:, h * 2 + which:h * 2 + which + 1],
                                nrm[:, base:base + NCH],
                                axis=mybir.AxisListType.X, op=OP.max)
                        cc = stat.tile([1, 1], f32, tag=f"cc{h}", name=f"cc{h}")
                        nc.vector.tensor_tensor(cc[:], mx[:, 2 * h:2 * h + 1],
                                                mx[:, 2 * h + 1:2 * h + 2],
                                                OP.mult)
                        nc.scalar.activation(cc[:], cc[:], AF.Sqrt)
                        nc.vector.tensor_scalar(cc[:], cc[:], -1.0, -0.5,
                                                OP.mult, OP.add)
                        nc.gpsimd.partition_broadcast(cbias[h][:], cc[:])

                    # pos = cumsum(sigmoid(y)); exact hi/lo split (fused
                    # magic-round; negation is exact so posR rows are just
                    # negated copies of the split)
                    nc.scalar.activation(y_sb[:], y_sb[:], AF.Sigmoid)
                    pos = posw.tile([HPC, SEQ], f32, tag="pos", name="pos")
                    nc.vector.tensor_tensor_scan(
                        pos[:], y_sb[:], y_sb[:], 0.0, OP.add, OP.bypass)
                    phi = posw.tile([HPC, SEQ], f32, tag="phi", name="phi")
                    nc.vector.tensor_scalar(phi[:], pos[:], C_ROUND, C_ROUND,
                                            OP.add, OP.subtract)
                    # pos becomes pos_lo in place (exact)
                    nc.vector.tensor_tensor(pos[:], pos[:], phi[:], OP.subtract)
                    nhi = posw.tile([HPC, SEQ], f32, tag="nhi", name="nhi")
                    nlo = posw.tile([HPC, SEQ], f32, tag="nlo", name="nlo")
                    nc.vector.tensor_scalar_mul(nhi[:], phi[:], -1.0)
                    nc.vector.tensor_scalar_mul(nlo[:], pos[:], -1.0)
                    for h in range(HPC):
                        zd1 = nc.sync.dma_start(posL[h][4:P, :],
                                                cpad_d.ap()[1:P - 3, :])
                        zd2 = nc.sync.dma_start(posR[h][4:P, :],
                                                cpad_d.ap()[1:P - 3, :])
                        add_dep(zd1.ins, xdmas[-1].ins, sync=True,
                                reason="zpad after x stream")
                        add_dep(zd2.ins, xdmas[-1].ins, sync=True,
                                reason="zpad after x stream")
                        nc.sync.dma_start(posL[h][0:1, :],
                                          phi[h:h + 1, :].bitcast(fmm))
                        nc.sync.dma_start(posL[h][1:2, :],
                                          pos[h:h + 1, :].bitcast(fmm))
                        nc.sync.dma_start(posL[h][2:3, :], cpad_d.ap()[0:1, :])
                        nc.sync.dma_start(posL[h][3:4, :], cpad_d.ap()[0:1, :])
                        nc.sync.dma_start(posR[h][0:1, :], cpad_d.ap()[0:1, :])
                        nc.sync.dma_start(posR[h][1:2, :], cpad_d.ap()[0:1, :])
                        nc.sync.dma_start(posR[h][2:3, :],
                                          nhi[h:h + 1, :].bitcast(fmm))
                        nc.sync.dma_start(posR[h][3:4, :],
                                          nlo[h:h + 1, :].bitcast(fmm))
                posw_cm.__exit__(None, None, None)

            # ================= stage D: attention =================
            late_cm = tc.tile_pool(name="late", bufs=1)
            late = late_cm.__enter__()  # closed after stage E (LIFO)
            # out-proj weights via SWDGE (gpsimd queue): the cbias
            # partition_broadcasts block that queue until end of stage C, so
            # these 9MB do not steal DMA bandwidth from x/weff early on
            wot_sb = []
            for kde in range(HEADS):
                w = late.tile([P, D_MODEL], fmm, tag=f"wot{kde}",
                              name=f"wot{kde}")
                wd = nc.sync.dma_start(w[:],
                                       wot_d.ap()[kde * P:(kde + 1) * P, :])
                add_dep(wd.ins, xdmas[-1].ins, sync=True,
                        reason="wot after x stream")
                wot_sb.append(w)
            wln = late.tile([P, D_MODEL], f32, tag="wln", name="wln")
            wd = nc.sync.dma_start(wln[:], wln_d.ap())
            add_dep(wd.ins, xdmas[-1].ins, sync=True, reason="wln after x")
            bln = late.tile([P, D_MODEL], f32, tag="bln", name="bln")
            wd = nc.sync.dma_start(bln[:], bln_d.ap())
            add_dep(wd.ins, xdmas[-1].ins, sync=True, reason="bln after x")

            with tc.tile_pool(name="psS", bufs=3, space="PSUM") as psS, \
                 tc.tile_pool(name="psO", bufs=2, space="PSUM") as psO, \
                 tc.tile_pool(name="psD", bufs=2, space="PSUM") as psD, \
                 tc.tile_pool(name="psR", bufs=1, space="PSUM") as psR, \
                 tc.tile_pool(name="pTp", bufs=6) as pTp, \
                 tc.tile_pool(name="zp", bufs=2) as zp:

                rdr128 = zp.tile([P, IC], fmm, tag="rdr128", name="rdr128",
                                 bufs=1)
                rd = nc.sync.dma_start(rdr128[1:P, :],
                                       cpad_d.ap()[1:P, 0:IC])
                add_dep(rd.ins, xdmas[-1].ins, sync=True,
                        reason="rdr128 pad after x stream")

                for h in range(HPC):
                    for ic in range(NCH):
                        o_pp = psO.tile([P, IC], f32, tag="opp", name="opp")
                        d_pp = psD.tile([P, IC], f32, tag="dpp", name="dpp")
                        njt = 4 * ic + 4
                        for jt in range(njt):
                            b = jt - 4 * ic
                            ioff = max(0, b) * P
                            N = IC - ioff
                            iabs = ic * IC + ioff
                            s_pp = psS.tile([P, IC], f32, tag="spp", name="spp")
                            nc.tensor.matmul(
                                s_pp[:, :N], kt_sb[h][:, jt * P:(jt + 1) * P],
                                q_sb[h][:, iabs:iabs + N], start=True, stop=False)
                            nc.tensor.matmul(
                                s_pp[:, :N], posL[h][:, jt * P:(jt + 1) * P],
                                posR[h][:, iabs:iabs + N],
                                start=False, stop=True)
                            if b >= 0:
                                # causal mask on the diagonal 128-block, added
                                # in PSUM before exp (garbage j>i entries can
                                # carry pos_j-pos_i up to +127 -> exp overflow)
                                nc.vector.tensor_tensor(s_pp[:, 0:P],
                                                        s_pp[:, 0:P],
                                                        mask[:], OP.add)
                            pT = pTp.tile([P, IC], fmm, tag="pT", name="pT")
                            nc.scalar.activation(pT[:, :N], s_pp[:, :N], AF.Exp,
                                                 bias=cbias[h][:])
                            nc.tensor.matmul(
                                o_pp[:, ioff:ioff + N], vT_sb[h][:, jt, :],
                                pT[:, :N], start=(jt == 0), stop=(jt == njt - 1),
                                skip_group_check=True)
                            nc.tensor.matmul(
                                d_pp[:, ioff:ioff + N], dsel[:], pT[:, :N],
                                start=(jt == 0), stop=(jt == njt - 1),
                                skip_group_check=True)
                        # epilogue: z = silu(p) * o / D for this chunk
                        # (D broadcast across partitions via a K=1 matmul,
                        #  then a true divide -- no limited-range reciprocal)
                        nc.vector.tensor_copy(rdr128[0:1, :], d_pp[0:1, :])
                        rb_pp = psR.tile([P, IC], f32, tag="rbpp", name="rbpp")
                        nc.tensor.matmul(rb_pp[:], rsel[:], rdr128[:],
                                         start=True, stop=True)
                        csl = slice(ic * IC, (ic + 1) * IC)
                        t1 = zp.tile([P, IC], f32, tag="t1", name="t1")
                        nc.vector.tensor_tensor(t1[:], o_pp[:],
                                                p_sb[h][:, csl], OP.mult)
                        rcp = zp.tile([P, IC], f32, tag="rcp", name="rcp")
                        nc.vector.reciprocal(rcp[:], rb_pp[:])
                        z_sb = zp.tile([P, IC], fmm, tag="z", name="z")
                        nc.vector.tensor_tensor(z_sb[:], t1[:], rcp[:],
                                                OP.mult)
                        dst = zin[h][:][2 * ic:2 * ic + 2, :, :] \
                            .rearrange("r p t -> p r t")
                        nc.sync.dma_start(
                            dst, z_sb[:].rearrange("p (r t) -> p r t", r=2))
                    # per-head AllToAll right after this head's chunks: the
                    # first exchange overlaps the second head's attention
                    nc.gpsimd.collective_compute(
                        "AllToAll", mybir.AluOpType.bypass,
                        replica_groups=[list(range(NC))],
                        ins=[zin[h][:].opt()], outs=[zout[h][:].opt()])

            # ========== stage E: out-projection + final LN ==========
            with tc.tile_pool(name="psE", bufs=2, space="PSUM") as psE, \
                 tc.tile_pool(name="zap", bufs=1) as zap, \
                 tc.tile_pool(name="outp", bufs=2) as outp:
                zall = {}
                for h in range(HPC):
                    for r in range(NC):
                        kde = 2 * r + h
                        zt = zap.tile([P, TS], fmm, tag=f"zall{kde}",
                                      name=f"zall{kde}")
                        nc.sync.dma_start(zt[:], zout[h][:][r, :, :])
                        zall[kde] = zt
                # accumulate h0 rows first (available after the first
                # AllToAll, overlapping the second), then h1 rows
                kde_order = [2 * r for r in range(NC)] + \
                    [2 * r + 1 for r in range(NC)]
                for ot in range(TS // P):
                    outf = outp.tile([P, D_MODEL], f32, tag="outf", name="outf")
                    for n in range(2):
                        opp2 = psE.tile([P, IC], f32, tag="oppE", name="oppE")
                        for ki, kde in enumerate(kde_order):
                            nc.tensor.matmul(
                                opp2[:], zall[kde][:, ot * P:(ot + 1) * P],
                                wot_sb[kde][:, n * IC:(n + 1) * IC],
                                start=(ki == 0), stop=(ki == HEADS - 1))
                        nc.scalar.copy(outf[:, n * IC:(n + 1) * IC], opp2[:])
                    # final layernorm over the 1024 features
                    bs2 = outp.tile([P, 12], f32, tag="bs2", name="bs2")
                    nc.vector.bn_stats(bs2[:, 0:6], outf[:, 0:512])
                    nc.vector.bn_stats(bs2[:, 6:12], outf[:, 512:1024])
                    mv2 = outp.tile([P, 2], f32, tag="mv2", name="mv2")
                    nc.vector.bn_aggr(mv2[:], bs2[:])
                    rs2 = outp.tile([P, 1], f32, tag="rs2", name="rs2")
                    nc.scalar.activation(rs2[:], mv2[:, 1:2], AF.Sqrt,
                                         bias=epsc[:])
                    nc.vector.reciprocal(rs2[:], rs2[:])
                    nm2 = outp.tile([P, 1], f32, tag="nm2", name="nm2")
                    nc.vector.tensor_tensor(nm2[:], mv2[:, 0:1], rs2[:], OP.mult)
                    nc.vector.tensor_scalar_mul(nm2[:], nm2[:], -1.0)
                    t2 = outp.tile([P, D_MODEL], f32, tag="t2", name="t2")
                    nc.scalar.activation(t2[:], outf[:], AF.Identity,
                                         bias=nm2[:], scale=rs2[:])
                    nc.vector.tensor_tensor(t2[:], t2[:], wln[:], OP.mult)
                    nc.vector.tensor_tensor(t2[:], t2[:], bln[:], OP.add)
                    nc.sync.dma_start(out_d.ap()[ot * P:(ot + 1) * P, :], t2[:])

            late_cm.__exit__(None, None, None)
            pers_cm.__exit__(None, None, None)

    nc.compile()
    return nc


def _get_program():
    if "prog" not in _CACHE:
        _CACHE["prog"] = _build_program(use_f32r=True)
    return _CACHE["prog"]


def _sigmoid(v):
    return 1.0 / (1.0 + np.exp(-v))


def kernel(x, W_in, b_in, in_ln_w, in_ln_b, W_out, out_ln_w, out_ln_b,
           smear_factor, log_scale):
    from concourse import bass_utils

    x = np.asarray(x, dtype=np.float32).reshape(SEQ, D_MODEL)
    W_in = np.asarray(W_in, dtype=np.float32)
    b_in = np.asarray(b_in, dtype=np.float32)
    in_ln_w = np.asarray(in_ln_w, dtype=np.float32)
    in_ln_b = np.asarray(in_ln_b, dtype=np.float32)
    W_out = np.asarray(W_out, dtype=np.float32)
    out_ln_w = np.asarray(out_ln_w, dtype=np.float32)
    out_ln_b = np.asarray(out_ln_b, dtype=np.float32)
    smear = _sigmoid(np.asarray(smear_factor, dtype=np.float64)).astype(np.float32)
    qscale = (np.exp(-2.0 * np.asarray(log_scale, dtype=np.float64))
              / np.sqrt(D_HEAD)).astype(np.float32)

    # fold the input layernorm affine into the projection
    WT = (W_in.T * in_ln_w[:, None]).astype(np.float32)      # [1024, 8208]
    b_eff = (b_in + in_ln_b @ W_in.T).astype(np.float32)     # [8208]

    wot = np.ascontiguousarray(W_out.T)                      # [2048, 1024]
    wln = np.broadcast_to(out_ln_w, (P, D_MODEL)).copy()
    bln = np.broadcast_to(out_ln_b, (P, D_MODEL)).copy()
    jj, ii = np.meshgrid(np.arange(P), np.arange(P), indexing="ij")
    masktri = np.where(jj <= ii, 0.0, -1.0e4).astype(np.float32)
    ident = np.eye(P, dtype=np.float32)
    dsel = np.zeros((P, P), dtype=np.float32)
    dsel[:, 0] = 1.0
    rsel = np.zeros((P, P), dtype=np.float32)
    rsel[0, :] = 1.0
    cpad = np.zeros((P, SEQ), dtype=np.float32)
    cpad[0, :] = 1.0

    in_maps = []
    for c in range(NC):
        h0 = HPC * c
        cols = []
        bcols = []
        for blk in range(4):  # q, k, v, p column blocks for this core's heads
            sl = WT[:, blk * D_EXP + h0 * D_HEAD:
                    blk * D_EXP + (h0 + HPC) * D_HEAD].copy()
            bsl = b_eff[blk * D_EXP + h0 * D_HEAD:
                        blk * D_EXP + (h0 + HPC) * D_HEAD].copy()
            if blk == 0:  # fold the 1/(s^2 sqrt(dh)) score scale into q
                for hh in range(HPC):
                    sl[:, hh * D_HEAD:(hh + 1) * D_HEAD] *= qscale[h0 + hh]
                    bsl[hh * D_HEAD:(hh + 1) * D_HEAD] *= qscale[h0 + hh]
            cols.append(sl)
            bcols.append(bsl)
        weff_c = np.zeros((D_MODEL, 8 * P + 2), dtype=np.float32)
        weff_c[:, :8 * P] = np.concatenate(cols, axis=1)
        weff_c[:, 8 * P:8 * P + HPC] = WT[:, 4 * D_EXP + h0:4 * D_EXP + h0 + HPC]
        beff_c = np.zeros((P, 9), dtype=np.float32)
        beff_c[:, :8] = np.concatenate(bcols).reshape(8, P).T
        beff_c[0:HPC, 8] = b_eff[4 * D_EXP + h0:4 * D_EXP + h0 + HPC]
        sm_c = np.zeros((P, 4), dtype=np.float32)
        sm_c[:, 0] = smear[h0]
        sm_c[:, 1] = 1.0 - smear[h0]
        sm_c[:, 2] = smear[h0 + 1]
        sm_c[:, 3] = 1.0 - smear[h0 + 1]
        in_maps.append({
            "x": x, "weff": weff_c, "beff": beff_c, "sm": sm_c,
            "wot": wot, "wln": wln, "bln": bln,
            "masktri": masktri, "ident": ident,
            "dsel": dsel, "rsel": rsel, "cpad": cpad,
        })

    nc = _get_program()
    trace = bool(int(os.environ.get("KERNEL_TRACE", "0")))
    res = bass_utils.run_bass_kernel_spmd(
        nc, in_maps, core_ids=list(range(NC)), trace=trace)
    _CACHE["last_results"] = res

    out = np.concatenate([res.results[c]["out"] for c in range(NC)], axis=0)
    return out.reshape(1, SEQ, D_MODEL)

